# revision 48
# baseline (speedup 1.0000x reference)
"""Trainium2 Bass kernel for JointIntegralRegressor (soft-argmax over 3D heatmaps).

reference math (per (n,j) volume V[d,h,w] of shape 64^3):
    p = softmax(V.flatten())
    x = sum(p * w)/W - 0.5 ; y = sum(p * h)/H - 0.5 ; z = sum(p * d)/D - 0.5

softmax is shift/scale-invariant in the E-ratios, so with E = exp(V - 3)
(the -3 keeps E inside fp8-e4m3 range; it cancels in every ratio):
    x = (sum w*E)/(sum E)/64 - 0.5   etc.

Design, driven by the memory roofline and measured engine rates:
  - HBM: host quantizes f32 heatmaps to int8 (q = round(20*V), clamped
    to [-36, 127]; the lower clamp keeps the bit-trick below from
    wrapping, and exp(-1.8-3) is below e4m3 resolution anyway). 4x less
    DMA than f32: ~12.6 MB/core ~= 37us at the ~340 GB/s per-core rate.
  - exp is the next wall (ScalarE ACT = 1 elem/cycle/lane): split across
    two engines at VOLUME granularity (per-volume softmax ratios cancel
    each engine's uniform bias):
      ScalarE: true exp via the ACT affine, E = Exp(q*0.05 - 3) -> e4m3
      VectorE: Schraudolph bit-trick: the e4m3 bit pattern of e^(x-3) is
        approx round(x*8/ln2 + 56 - 24/ln2), one tensor_scalar
        (q*A + B) -> uint8 alias of the e4m3 tile (2 elem/cycle/lane).
    Per-element error is a +-6% sawtooth; it cancels to ~1.2e-4 in the
    coordinate ratios (verified on host in f64 against the reference).
  - TensorE: fp8 DoubleRow matmul (2 contraction rows/cycle) reduces
    partitions at 614 G elem/s. The pair k-tiles are the interleaved
    even/odd free columns, so each PSUM output column n is a weighted
    PAIR sum over e-cols (2n, 2n+1) - halving PSUM width and stage 2.

Per-core layout: a volume is 256 KiB int8 -> SBUF [128, 2048] where
    partition p: d = p>>1, hpar = p&1   (h = 32*hpar + (f>>6))
    free f:      j = f>>6 (h low bits), w = f&63
Weights per volume (block-diagonal; 16 volumes/round, rows 4s..):
    row s: ones (w-target), 16+s: dmid = p>>3, 32+s: dlo2 + 4*hpar,
    48+s: dlo2 - 4*hpar   (dlo2 = (p>>1)&3; d = 8*dmid... no: d =
    4*dmid + dlo2). All weights are small ints, exact in e4m3.
Stage 2 is a single ScalarE pass per round: ACT Copy of the PSUM strip
[64, 1024] -> SBUF with accum_out = per-row plain sums. That yields
    S (row s), DmidE (16+s), A (32+s), B (48+s)
directly; rows 0:16 of the copied strip (the per-pair column sums of
each volume) are DMA'd to the host, which computes the w- and j-
weighted sums in numpy (f64, exact):
    XE = sum_n 2*(n&31)*strip[n] + S/2   (pairs merge w and w+1; the
         S/2 splits each pair evenly - adds ~3e-5 error)
    JE = sum_n ((2n)>>6)*strip[n]        (pairs share one j - exact)
    ZE = 4*DmidE + (A+B)/2 ; ParE = (A-B)/8
    x = XE/S/64-0.5, y = (32*ParE+JE)/S/64-0.5, z = ZE/S/64-0.5
VectorE does NO stage-2 work and GpSimd only issues two mid-kernel
result stores, so both stay available for exp / DMA.
"""

import sys

if "/opt/trn_rl_repo" not in sys.path:
    sys.path.insert(0, "/opt/trn_rl_repo")

from contextlib import ExitStack

import ml_dtypes
import numpy as np

import concourse.bass as bass
import concourse.tile as tile
from concourse import bacc, mybir
from concourse.bass_utils import run_bass_kernel_spmd

N, J, D, H, W = 16, 24, 64, 64, 64
VOLS = N * J  # 384
NCORES = 8
VPC = VOLS // NCORES  # 48 volumes per core
P = 128
F = 2048  # free elems per partition per volume (64^3 / 128)
RVOL = 16  # volumes per PSUM round
NR = VPC // RVOL  # 3 rounds
FP = F // 2  # pair columns per volume

QSCALE = 20.0  # int8 quant step: q = round(QSCALE * x)
QLO = -36  # lower clamp: keeps the uint8 bit-trick non-negative
# Schraudolph constants: e4m3 bits of e^(q/QSCALE - 3) ~ q*A8 + B8
SCHRA_A8 = 8.0 / (QSCALE * np.log(2.0))
SCHRA_B8 = 56.0 - 24.0 / np.log(2.0) - 0.25

# exp engine per volume: 'S' = ScalarE true exp, 'D' = VectorE bit-trick.
# ScalarE measures ~1.92us/vol + ~1.2us/round of stage-2; DVE ~1.2us/vol
# -> 17 S / 31 D puts both near the ~37us DMA floor. D-volumes sit
# contiguous at the tail of each 4-vol DMA batch so one batched
# tensor_scalar covers them.
GROUPS = ["SD" if i % 3 != 2 else "DD" for i in range(22)]
ENGINES = list("SDSD") + [c for g in GROUPS for c in g]
# last pair runs D-then-S: volume 46's matmuls then chase the faster
# DVE exp while ScalarE's 1.9us exp for volume 47 overlaps them,
# shortening the post-stream tail chain by ~0.6us
ENGINES[-2:] = ["D", "S"]
assert len(ENGINES) == VPC and ENGINES.count("S") == 17

STAGE2_DELAY = 5  # volumes between a round's last matmul and its stage 2

_cache = {}


def _stage2(nc, scratch, res, prs, strips, accs, r):
    """One ScalarE pass per finished round: ACT Copy of the PSUM strip
    [64, FP] to SBUF with accum_out = plain row sums. Rows 0:16 of the
    copy (each volume's per-pair column sums) and the 64 accumulators
    go to the host, which does the w-/j-weighted sums in f64. Mid-
    kernel stores ride the scalar ring right behind the Copy that
    produced them (GpSimd SWDGE costs ~1us descriptor-gen + drain per
    store); the final round rides the sync ring, empty by then."""
    # strip in bf16: ScalarE 16-bit output runs at 2 elem/cycle (the
    # accumulators stay f32); the per-column 0.4% rounding noise adds
    # only ~6e-5 to the coordinate error (host-verified)
    strip = scratch.tile([P, FP], mybir.dt.bfloat16, tag="strip")
    acc = res.tile([P, 1], mybir.dt.float32, tag="acc")
    nc.scalar.activation(
        strip[0:64, :],
        prs[r][0:64, :],
        mybir.ActivationFunctionType.Copy,
        accum_out=acc[0:64, :],
    )
    # NB: final-round stores via GpSimd SWDGE crashed the exec unit
    # (NRT_EXEC_UNIT_UNRECOVERABLE) - keep them on the sync HWDGE ring
    eng = nc.sync if r == NR - 1 else nc.scalar
    eng.dma_start(strips[r], strip[0:RVOL, :])
    eng.dma_start(accs[r], acc[0:64, :])


def _build():
    nc = bacc.Bacc("TRN2", target_bir_lowering=False, debug=False)
    # partition-major layout (host pre-transposes): a 4-volume batch is
    # then 8 KB CONTIGUOUS per partition row. The [vol, p, f] layout's
    # strided gather (2 KB runs at 256 KB stride) measured ~310 GB/s vs
    # ~340 contiguous.
    heat = nc.dram_tensor(
        "heat", [P, VPC, F], mybir.dt.int8, kind="ExternalInput"
    ).ap()
    # host-built block-diagonal stage-1 weights, 256 KB of e4m3: volume
    # slot s uses block [:, 128s:128(s+1)] = [Ws | Ws] (the DoubleRow
    # k-tile halves carry identical weights so both pair elements get
    # the same coefficient). PE requires matmul outputs at base
    # partition 0, so every slot writes all 64 output rows and lands its
    # 4 rows via its own weight columns; the 16 slots of a round
    # accumulate into one PSUM tile.
    w1blk = nc.dram_tensor(
        "w1blk", [P, 128 * RVOL], mybir.dt.float8e4, kind="ExternalInput"
    ).ap()
    strips = nc.dram_tensor(
        "strips", [NR, RVOL, FP], mybir.dt.bfloat16, kind="ExternalOutput"
    ).ap()
    accs = nc.dram_tensor(
        "accs", [NR, 64, 1], mybir.dt.float32, kind="ExternalOutput"
    ).ap()

    with tile.TileContext(nc) as tc, ExitStack() as ctx:
        const = ctx.enter_context(tc.tile_pool(name="const", bufs=1))
        raws = ctx.enter_context(tc.tile_pool(name="raw", bufs=9))
        es = ctx.enter_context(tc.tile_pool(name="e", bufs=8))
        psums = ctx.enter_context(
            tc.tile_pool(name="ps", bufs=1, space=bass.MemorySpace.PSUM)
        )
        scratch = ctx.enter_context(tc.tile_pool(name="scr", bufs=2))
        res = ctx.enter_context(tc.tile_pool(name="res", bufs=2))

        # w1 on the scalar HWDGE ring, slot-0 block first so the first
        # matmul ungates after ~2 KB (the sync ring stays exclusively
        # heat loads: HWDGE is FIFO per issuing engine; loading heat on
        # the scalar ring instead measured NO earlier arrival and a
        # store-ordering penalty)
        w1_t = const.tile([P, 128 * RVOL], mybir.dt.float8e4)
        nc.scalar.dma_start(w1_t[:, 0:128], w1blk[:, 0:128])
        nc.scalar.dma_start(w1_t[:, 128:], w1blk[:, 128:])
        # per-partition bias column for the Exp affine (a float bias
        # would need a pre-registered const AP)
        bias3 = const.tile([P, 1], mybir.dt.float32)
        nc.gpsimd.memset(bias3[:], -3.0)

        # volume load batches: 1 MiB quads in the steady state (a single
        # dma_start is split across all 16 SDMA engines; >=1 MiB is
        # needed for full HBM bandwidth), but 256 KiB singles at the
        # very start (first exp starts sooner) and very end (the last
        # in-flight loads complete in a burst, so smaller grains cut the
        # trailing exp backlog after the final DMA lands)
        # 256 KiB singles for the first volumes (first exp starts
        # sooner), then 512 KiB pairs: small enough that the exp
        # engines drafting right behind the saturated stream wait
        # ~1.5us per batch completion instead of ~3us, large enough
        # (4 KB contiguous per partition) for full DMA efficiency
        batches = [(0, 1), (1, 1), (2, 1), (3, 1)] + [
            (g, 2) for g in range(4, VPC, 2)
        ]

        def exp_scalar(e, raw, k):
            # E = exp(q/QSCALE - 3) via the ACT affine, one op per vol
            nc.scalar.activation(
                e[:, k * F : (k + 1) * F],
                raw[:, k * F : (k + 1) * F],
                mybir.ActivationFunctionType.Exp,
                bias=bias3[:],
                scale=1.0 / QSCALE,
            )

        def exp_vector(e, raw, c0, c1):
            # Schraudolph: e4m3 bits of e^(q/QSCALE - 3) ~ q*A8 + B8,
            # one batched tensor_scalar over cols [c0, c1) -> uint8
            # alias. The f32->uint8 output conversion WRAPS mod 256 on
            # negatives (measured), which the host-side q >= QLO clamp
            # makes unreachable.
            ei = e[:, c0:c1].bitcast(mybir.dt.uint8)
            nc.vector.tensor_scalar(
                ei,
                raw[:, c0:c1],
                SCHRA_A8,
                SCHRA_B8,
                mybir.AluOpType.mult,
                mybir.AluOpType.add,
            )

        prs = [
            psums.tile([P, FP], mybir.dt.float32, name=f"pr{r}") for r in range(NR)
        ]
        for g0, nv in batches:
            # all loads on the single sync HWDGE ring: consumption is
            # strictly in volume order, so splitting loads across queues
            # reorders arrivals and stalls every engine on the laggard
            # (measured as a 9.5us whole-pipeline bubble)
            raw = raws.tile([P, nv * F], mybir.dt.int8, tag="raw")
            nc.sync.dma_start(
                raw[:].rearrange("p (v f) -> p v f", v=nv),
                heat[:, g0 : g0 + nv, :],
            )
            e = es.tile([P, nv * F], mybir.dt.float8e4, tag="e")
            # issue exp per engine: ScalarE vols one op each, the
            # contiguous DVE tail of the batch as one batched op (split
            # in two for the very last volume so its matmuls chase the
            # exp chunks)
            kd = [k for k in range(nv) if ENGINES[g0 + k] == "D"]
            for k in range(nv):
                if ENGINES[g0 + k] == "S":
                    exp_scalar(e, raw, k)
            if kd:
                assert kd == list(range(kd[0], kd[0] + len(kd)))
                if g0 + kd[-1] == VPC - 1:
                    exp_vector(e, raw, kd[0] * F, kd[-1] * F + F // 2)
                    exp_vector(e, raw, kd[-1] * F + F // 2, (kd[-1] + 1) * F)
                else:
                    exp_vector(e, raw, kd[0] * F, (kd[-1] + 1) * F)
            for k in range(nv):
                v = g0 + k
                r, s = divmod(v, RVOL)
                pr = prs[r]
                for b in range(4):
                    nc.tensor.matmul(
                        pr[0:64, 256 * b : 256 * (b + 1)],
                        w1_t[:, 128 * s : 128 * (s + 1)].rearrange(
                            "p (two m) -> p two m", two=2
                        ),
                        e[:, k * F + 512 * b : k * F + 512 * (b + 1)].rearrange(
                            "p (f two) -> p two f", two=2
                        ),
                        # 256-f32 chunks pair up within 2 KB PSUM banks:
                        # start/stop once per bank (start zeroes the
                        # whole zero region; the second chunk accums)
                        start=(s == 0 and b % 2 == 0),
                        stop=(s == RVOL - 1 and b % 2 == 1),
                        perf_mode=mybir.MatmulPerfMode.DoubleRow,
                    )
                # stage 2 for round r is emitted STAGE2_DELAY volumes
                # after the round closes: it sits in ScalarE's in-order
                # queue, and emitting it right at the boundary made
                # ScalarE block on the round's last matmul for ~5us
                # instead of running the next volumes' exps
                if v >= RVOL - 1 + STAGE2_DELAY and (v - STAGE2_DELAY) % RVOL == RVOL - 1:
                    _stage2(nc, scratch, res, prs, strips, accs, (v - STAGE2_DELAY) // RVOL)
        _stage2(nc, scratch, res, prs, strips, accs, NR - 1)

    nc.compile()
    return nc


def _host_inputs():
    p = np.arange(P)
    dmid = p >> 3
    dlo2 = (p >> 1) & 3
    par = p & 1
    ws = np.zeros((P, 64), dtype=np.float32)
    w1 = np.zeros((P, 128 * RVOL), dtype=np.float32)
    for s in range(RVOL):
        ws[:] = 0.0
        ws[:, s] = 1.0
        ws[:, 16 + s] = dmid
        ws[:, 32 + s] = dlo2 + 4 * par
        ws[:, 48 + s] = dlo2 - 4.0 * par
        w1[:, 128 * s : 128 * s + 64] = ws
        w1[:, 128 * s + 64 : 128 * s + 128] = ws
    return w1.astype(ml_dtypes.float8_e4m3)


def _quantize(heatmaps):
    """f32 [N,J,D,H,W] -> int8 [NCORES, P, VPC, F] (q = round(QSCALE*x)),
    partition-major per core so device loads are contiguous."""
    x = np.asarray(heatmaps, dtype=np.float32).reshape(NCORES, VPC, P, F)
    q = np.clip(np.rint(x * QSCALE), QLO, 127).astype(np.int8)
    return np.ascontiguousarray(q.transpose(0, 2, 1, 3))


def _decode(results):
    """results: 8 dicts with strips [NR,16,FP], accs [NR,64,1] -> [16,24,3]."""
    g = np.arange(FP)
    wpat = 2.0 * (g & 31)
    jpat = ((2 * g) >> 6).astype(np.float64)
    preds = np.empty((NCORES * VPC, 3))
    for ci, rr in enumerate(results):
        strip = np.asarray(rr["strips"]).astype(np.float64)  # [NR, 16, FP]
        acc = np.asarray(rr["accs"]).astype(np.float64).reshape(NR, 64)
        S = acc[:, 0:16]
        DmidE = acc[:, 16:32]
        A = acc[:, 32:48]
        B = acc[:, 48:64]
        XE = (strip * wpat).sum(-1) + 0.5 * S
        JE = (strip * jpat).sum(-1)
        ZE = 4.0 * DmidE + (A + B) / 2.0
        ParE = (A - B) / 8.0
        x = XE / S / W - 0.5
        y = (32.0 * ParE + JE) / S / H - 0.5
        z = ZE / S / D - 0.5
        preds[ci * VPC : (ci + 1) * VPC] = np.stack(
            [x.ravel(), y.ravel(), z.ravel()], axis=1
        )
    return preds.astype(np.float32).reshape(N, J, 3)


def kernel(heatmaps, **run_kwargs):
    assert np.asarray(heatmaps).shape == (N, J, D, H, W)
    if "nc" not in _cache:
        _cache["nc"] = _build()
    nc = _cache["nc"]
    heat = _quantize(heatmaps)
    w1blk = _host_inputs()
    in_maps = [
        {"heat": heat[c], "w1blk": w1blk} for c in range(NCORES)
    ]
    res = run_bass_kernel_spmd(
        nc, in_maps, core_ids=list(range(NCORES)), **run_kwargs
    )
    preds = _decode(res.results)
    if run_kwargs:
        _cache["last_results"] = res
    return preds


# revision 50
# speedup vs baseline: 1.0400x; 1.0400x over previous
"""Trainium2 Bass kernel for JointIntegralRegressor (soft-argmax over 3D heatmaps).

reference math (per (n,j) volume V[d,h,w] of shape 64^3):
    p = softmax(V.flatten())
    x = sum(p * w)/W - 0.5 ; y = sum(p * h)/H - 0.5 ; z = sum(p * d)/D - 0.5

softmax is shift/scale-invariant in the E-ratios, so with E = exp(V - 3)
(the -3 keeps E inside fp8-e4m3 range; it cancels in every ratio):
    x = (sum w*E)/(sum E)/64 - 0.5   etc.

Design, driven by the memory roofline and measured engine rates:
  - HBM: host quantizes f32 heatmaps to int8 (q = round(20*V), clamped
    to [-36, 127]; the lower clamp keeps the bit-trick below from
    wrapping, and exp(-1.8-3) is below e4m3 resolution anyway). 4x less
    DMA than f32: ~12.6 MB/core ~= 37us at the ~340 GB/s per-core rate.
  - exp is the next wall (ScalarE ACT = 1 elem/cycle/lane): split across
    two engines at VOLUME granularity (per-volume softmax ratios cancel
    each engine's uniform bias):
      ScalarE: true exp via the ACT affine, E = Exp(q*0.05 - 3) -> e4m3
      VectorE: Schraudolph bit-trick: the e4m3 bit pattern of e^(x-3) is
        approx round(x*8/ln2 + 56 - 24/ln2), one tensor_scalar
        (q*A + B) -> uint8 alias of the e4m3 tile (2 elem/cycle/lane).
    Per-element error is a +-6% sawtooth; it cancels to ~1.2e-4 in the
    coordinate ratios (verified on host in f64 against the reference).
  - TensorE: fp8 DoubleRow matmul (2 contraction rows/cycle) reduces
    partitions at 614 G elem/s. The pair k-tiles are the interleaved
    even/odd free columns, so each PSUM output column n is a weighted
    PAIR sum over e-cols (2n, 2n+1) - halving PSUM width and stage 2.

Per-core layout: a volume is 256 KiB int8 -> SBUF [128, 2048] where
    partition p: d = p>>1, hpar = p&1   (h = 32*hpar + (f>>6))
    free f:      j = f>>6 (h low bits), w = f&63
Weights per volume (block-diagonal; 16 volumes/round, rows 4s..):
    row s: ones (w-target), 16+s: dmid = p>>3, 32+s: dlo2 + 4*hpar,
    48+s: dlo2 - 4*hpar   (dlo2 = (p>>1)&3; d = 8*dmid... no: d =
    4*dmid + dlo2). All weights are small ints, exact in e4m3.
Stage 2 is a single ScalarE pass per round: ACT Copy of the PSUM strip
[64, 1024] -> SBUF with accum_out = per-row plain sums. That yields
    S (row s), DmidE (16+s), A (32+s), B (48+s)
directly; rows 0:16 of the copied strip (the per-pair column sums of
each volume) are DMA'd to the host, which computes the w- and j-
weighted sums in numpy (f64, exact):
    XE = sum_n 2*(n&31)*strip[n] + S/2   (pairs merge w and w+1; the
         S/2 splits each pair evenly - adds ~3e-5 error)
    JE = sum_n ((2n)>>6)*strip[n]        (pairs share one j - exact)
    ZE = 4*DmidE + (A+B)/2 ; ParE = (A-B)/8
    x = XE/S/64-0.5, y = (32*ParE+JE)/S/64-0.5, z = ZE/S/64-0.5
VectorE does NO stage-2 work and GpSimd only issues two mid-kernel
result stores, so both stay available for exp / DMA.
"""

import sys

if "/opt/trn_rl_repo" not in sys.path:
    sys.path.insert(0, "/opt/trn_rl_repo")

from contextlib import ExitStack

import ml_dtypes
import numpy as np

import concourse.bass as bass
import concourse.tile as tile
from concourse import bacc, mybir
from concourse.bass_utils import run_bass_kernel_spmd

N, J, D, H, W = 16, 24, 64, 64, 64
VOLS = N * J  # 384
NCORES = 8
VPC = VOLS // NCORES  # 48 volumes per core
P = 128
F = 2048  # free elems per partition per volume (64^3 / 128)
RVOL = 16  # volumes per PSUM round
NR = VPC // RVOL  # 3 rounds
FP = F // 2  # pair columns per volume

QSCALE = 20.0  # int8 quant step: q = round(QSCALE * x)
QLO = -36  # lower clamp: keeps the uint8 bit-trick non-negative
# Schraudolph constants: e4m3 bits of e^(q/QSCALE - 3) ~ q*A8 + B8
SCHRA_A8 = 8.0 / (QSCALE * np.log(2.0))
SCHRA_B8 = 56.0 - 24.0 / np.log(2.0) - 0.25

# exp engine per volume: 'S' = ScalarE true exp, 'D' = VectorE bit-trick.
# ScalarE measures ~1.92us/vol + ~1.2us/round of stage-2; DVE ~1.2us/vol
# -> 17 S / 31 D puts both near the ~37us DMA floor. D-volumes sit
# contiguous at the tail of each 4-vol DMA batch so one batched
# tensor_scalar covers them.
GROUPS = ["SD" if i % 3 != 2 else "DD" for i in range(22)]
ENGINES = list("SDSD") + [c for g in GROUPS for c in g]
# last pair runs D-then-S: volume 46's matmuls then chase the faster
# DVE exp while ScalarE's 1.9us exp for volume 47 overlaps them,
# shortening the post-stream tail chain by ~0.6us
ENGINES[-2:] = ["D", "S"]
assert len(ENGINES) == VPC and ENGINES.count("S") == 17

STAGE2_DELAY = 5  # volumes between a round's last matmul and its stage 2

_cache = {}


def _stage2(nc, scratch, res, prs, strips, accs, r):
    """One ScalarE pass per finished round: ACT Copy of the PSUM strip
    [64, FP] to SBUF with accum_out = plain row sums. Rows 0:16 of the
    copy (each volume's per-pair column sums) and the 64 accumulators
    go to the host, which does the w-/j-weighted sums in f64. Mid-
    kernel stores ride the scalar ring right behind the Copy that
    produced them (GpSimd SWDGE costs ~1us descriptor-gen + drain per
    store); the final round rides the sync ring, empty by then."""
    strip = scratch.tile([P, FP], mybir.dt.float32, tag="strip")
    acc = res.tile([P, 1], mybir.dt.float32, tag="acc")
    nc.scalar.activation(
        strip[0:64, :],
        prs[r][0:64, :],
        mybir.ActivationFunctionType.Copy,
        accum_out=acc[0:64, :],
    )
    # NB: final-round stores via GpSimd SWDGE crashed the exec unit
    # (NRT_EXEC_UNIT_UNRECOVERABLE) - keep them on the sync HWDGE ring
    eng = nc.sync if r == NR - 1 else nc.scalar
    eng.dma_start(strips[r], strip[0:RVOL, :])
    eng.dma_start(accs[r], acc[0:64, :])


def _build():
    nc = bacc.Bacc("TRN2", target_bir_lowering=False, debug=False)
    # partition-major layout (host pre-transposes): a 4-volume batch is
    # then 8 KB CONTIGUOUS per partition row. The [vol, p, f] layout's
    # strided gather (2 KB runs at 256 KB stride) measured ~310 GB/s vs
    # ~340 contiguous.
    heat = nc.dram_tensor(
        "heat", [P, VPC, F], mybir.dt.int8, kind="ExternalInput"
    ).ap()
    # host-built block-diagonal stage-1 weights, 256 KB of e4m3: volume
    # slot s uses block [:, 128s:128(s+1)] = [Ws | Ws] (the DoubleRow
    # k-tile halves carry identical weights so both pair elements get
    # the same coefficient). PE requires matmul outputs at base
    # partition 0, so every slot writes all 64 output rows and lands its
    # 4 rows via its own weight columns; the 16 slots of a round
    # accumulate into one PSUM tile.
    w1blk = nc.dram_tensor(
        "w1blk", [P, 128 * RVOL], mybir.dt.float8e4, kind="ExternalInput"
    ).ap()
    strips = nc.dram_tensor(
        "strips", [NR, RVOL, FP], mybir.dt.float32, kind="ExternalOutput"
    ).ap()
    accs = nc.dram_tensor(
        "accs", [NR, 64, 1], mybir.dt.float32, kind="ExternalOutput"
    ).ap()

    with tile.TileContext(nc) as tc, ExitStack() as ctx:
        const = ctx.enter_context(tc.tile_pool(name="const", bufs=1))
        raws = ctx.enter_context(tc.tile_pool(name="raw", bufs=9))
        es = ctx.enter_context(tc.tile_pool(name="e", bufs=8))
        psums = ctx.enter_context(
            tc.tile_pool(name="ps", bufs=1, space=bass.MemorySpace.PSUM)
        )
        scratch = ctx.enter_context(tc.tile_pool(name="scr", bufs=2))
        res = ctx.enter_context(tc.tile_pool(name="res", bufs=2))

        # w1 on the scalar HWDGE ring, slot-0 block first so the first
        # matmul ungates after ~2 KB (the sync ring stays exclusively
        # heat loads: HWDGE is FIFO per issuing engine; loading heat on
        # the scalar ring instead measured NO earlier arrival and a
        # store-ordering penalty)
        w1_t = const.tile([P, 128 * RVOL], mybir.dt.float8e4)
        nc.scalar.dma_start(w1_t[:, 0:128], w1blk[:, 0:128])
        nc.scalar.dma_start(w1_t[:, 128:], w1blk[:, 128:])
        # per-partition bias column for the Exp affine (a float bias
        # would need a pre-registered const AP)
        bias3 = const.tile([P, 1], mybir.dt.float32)
        nc.gpsimd.memset(bias3[:], -3.0)

        # volume load batches: 1 MiB quads in the steady state (a single
        # dma_start is split across all 16 SDMA engines; >=1 MiB is
        # needed for full HBM bandwidth), but 256 KiB singles at the
        # very start (first exp starts sooner) and very end (the last
        # in-flight loads complete in a burst, so smaller grains cut the
        # trailing exp backlog after the final DMA lands)
        # 256 KiB singles for the first volumes (first exp starts
        # sooner), then 512 KiB pairs: small enough that the exp
        # engines drafting right behind the saturated stream wait
        # ~1.5us per batch completion instead of ~3us, large enough
        # (4 KB contiguous per partition) for full DMA efficiency
        batches = [(0, 1), (1, 1), (2, 1), (3, 1)] + [
            (g, 2) for g in range(4, VPC, 2)
        ]

        def exp_scalar(e, raw, k):
            # E = exp(q/QSCALE - 3) via the ACT affine, one op per vol
            nc.scalar.activation(
                e[:, k * F : (k + 1) * F],
                raw[:, k * F : (k + 1) * F],
                mybir.ActivationFunctionType.Exp,
                bias=bias3[:],
                scale=1.0 / QSCALE,
            )

        def exp_vector(e, raw, c0, c1):
            # Schraudolph: e4m3 bits of e^(q/QSCALE - 3) ~ q*A8 + B8,
            # one batched tensor_scalar over cols [c0, c1) -> uint8
            # alias. The f32->uint8 output conversion WRAPS mod 256 on
            # negatives (measured), which the host-side q >= QLO clamp
            # makes unreachable.
            ei = e[:, c0:c1].bitcast(mybir.dt.uint8)
            nc.vector.tensor_scalar(
                ei,
                raw[:, c0:c1],
                SCHRA_A8,
                SCHRA_B8,
                mybir.AluOpType.mult,
                mybir.AluOpType.add,
            )

        prs = [
            psums.tile([P, FP], mybir.dt.float32, name=f"pr{r}") for r in range(NR)
        ]
        for g0, nv in batches:
            # all loads on the single sync HWDGE ring: consumption is
            # strictly in volume order, so splitting loads across queues
            # reorders arrivals and stalls every engine on the laggard
            # (measured as a 9.5us whole-pipeline bubble)
            raw = raws.tile([P, nv * F], mybir.dt.int8, tag="raw")
            nc.sync.dma_start(
                raw[:].rearrange("p (v f) -> p v f", v=nv),
                heat[:, g0 : g0 + nv, :],
            )
            e = es.tile([P, nv * F], mybir.dt.float8e4, tag="e")
            # issue exp per engine: ScalarE vols one op each, the
            # contiguous DVE tail of the batch as one batched op (split
            # in two for the very last volume so its matmuls chase the
            # exp chunks)
            kd = [k for k in range(nv) if ENGINES[g0 + k] == "D"]
            for k in range(nv):
                if ENGINES[g0 + k] == "S":
                    exp_scalar(e, raw, k)
            if kd:
                assert kd == list(range(kd[0], kd[0] + len(kd)))
                if g0 + kd[-1] == VPC - 1:
                    exp_vector(e, raw, kd[0] * F, kd[-1] * F + F // 2)
                    exp_vector(e, raw, kd[-1] * F + F // 2, (kd[-1] + 1) * F)
                else:
                    exp_vector(e, raw, kd[0] * F, (kd[-1] + 1) * F)
            for k in range(nv):
                v = g0 + k
                r, s = divmod(v, RVOL)
                pr = prs[r]
                for b in range(4):
                    nc.tensor.matmul(
                        pr[0:64, 256 * b : 256 * (b + 1)],
                        w1_t[:, 128 * s : 128 * (s + 1)].rearrange(
                            "p (two m) -> p two m", two=2
                        ),
                        e[:, k * F + 512 * b : k * F + 512 * (b + 1)].rearrange(
                            "p (f two) -> p two f", two=2
                        ),
                        # 256-f32 chunks pair up within 2 KB PSUM banks:
                        # start/stop once per bank (start zeroes the
                        # whole zero region; the second chunk accums)
                        start=(s == 0 and b % 2 == 0),
                        stop=(s == RVOL - 1 and b % 2 == 1),
                        perf_mode=mybir.MatmulPerfMode.DoubleRow,
                    )
                # stage 2 for round r is emitted STAGE2_DELAY volumes
                # after the round closes: it sits in ScalarE's in-order
                # queue, and emitting it right at the boundary made
                # ScalarE block on the round's last matmul for ~5us
                # instead of running the next volumes' exps
                if v >= RVOL - 1 + STAGE2_DELAY and (v - STAGE2_DELAY) % RVOL == RVOL - 1:
                    _stage2(nc, scratch, res, prs, strips, accs, (v - STAGE2_DELAY) // RVOL)
        _stage2(nc, scratch, res, prs, strips, accs, NR - 1)

    nc.compile()
    return nc


def _host_inputs():
    p = np.arange(P)
    dmid = p >> 3
    dlo2 = (p >> 1) & 3
    par = p & 1
    ws = np.zeros((P, 64), dtype=np.float32)
    w1 = np.zeros((P, 128 * RVOL), dtype=np.float32)
    for s in range(RVOL):
        ws[:] = 0.0
        ws[:, s] = 1.0
        ws[:, 16 + s] = dmid
        ws[:, 32 + s] = dlo2 + 4 * par
        ws[:, 48 + s] = dlo2 - 4.0 * par
        w1[:, 128 * s : 128 * s + 64] = ws
        w1[:, 128 * s + 64 : 128 * s + 128] = ws
    return w1.astype(ml_dtypes.float8_e4m3)


def _quantize(heatmaps):
    """f32 [N,J,D,H,W] -> int8 [NCORES, P, VPC, F] (q = round(QSCALE*x)),
    partition-major per core so device loads are contiguous."""
    x = np.asarray(heatmaps, dtype=np.float32).reshape(NCORES, VPC, P, F)
    q = np.clip(np.rint(x * QSCALE), QLO, 127).astype(np.int8)
    return np.ascontiguousarray(q.transpose(0, 2, 1, 3))


def _decode(results):
    """results: 8 dicts with strips [NR,16,FP], accs [NR,64,1] -> [16,24,3]."""
    g = np.arange(FP)
    wpat = 2.0 * (g & 31)
    jpat = ((2 * g) >> 6).astype(np.float64)
    preds = np.empty((NCORES * VPC, 3))
    for ci, rr in enumerate(results):
        strip = np.asarray(rr["strips"]).astype(np.float64)  # [NR, 16, FP]
        acc = np.asarray(rr["accs"]).astype(np.float64).reshape(NR, 64)
        S = acc[:, 0:16]
        DmidE = acc[:, 16:32]
        A = acc[:, 32:48]
        B = acc[:, 48:64]
        XE = (strip * wpat).sum(-1) + 0.5 * S
        JE = (strip * jpat).sum(-1)
        ZE = 4.0 * DmidE + (A + B) / 2.0
        ParE = (A - B) / 8.0
        x = XE / S / W - 0.5
        y = (32.0 * ParE + JE) / S / H - 0.5
        z = ZE / S / D - 0.5
        preds[ci * VPC : (ci + 1) * VPC] = np.stack(
            [x.ravel(), y.ravel(), z.ravel()], axis=1
        )
    return preds.astype(np.float32).reshape(N, J, 3)


def kernel(heatmaps, **run_kwargs):
    assert np.asarray(heatmaps).shape == (N, J, D, H, W)
    if "nc" not in _cache:
        _cache["nc"] = _build()
    nc = _cache["nc"]
    heat = _quantize(heatmaps)
    w1blk = _host_inputs()
    in_maps = [
        {"heat": heat[c], "w1blk": w1blk} for c in range(NCORES)
    ]
    res = run_bass_kernel_spmd(
        nc, in_maps, core_ids=list(range(NCORES)), **run_kwargs
    )
    preds = _decode(res.results)
    if run_kwargs:
        _cache["last_results"] = res
    return preds
